# revision 60
# baseline (speedup 1.0000x reference)
"""Trainium2 Bass kernel for ClassicAttention (B=2, S=2048, D=1024, H=16).

Sharding: tensor-parallel over heads across 8 cores (2 heads/core), with
NO on-device collectives: each core computes a partial c_proj output from
its own heads' context (c_proj input rows are head dims, per the TP-head
sharding), DMAs the [1024, M] bf16 partial to DRAM, and the host sums the
8 partials during unsharding.  Profiling showed the collective engine
costs ~30us per ctx AllGather when overlapped with compute (~120us total)
plus an exposed tail; the partial-sum form replaces all of that with
~25us of fully-overlapped output DMA and a small tail.

  - Host pre-transposes x to x^T [D, M] and pre-casts all matmul operands
    to bf16; broadcast bias tiles are built on-device with K=1 matmuls from
    a 1KB bias row (no seed matmuls in the steady state, bias fused into
    PSUM-evacuation adds).
  - QKV: each core computes Q^T,K^T (d-major) and V (row-major) for its 2
    heads over all B*S rows straight from x^T in SBUF.
  - Attention: transposed-scores formulation S^T[k,q]; both heads share one
    [128,1024] score tile so each k-tile needs a single exp ACTIVATE.  The
    softmax denominator rides row 64 of the AV accumulator via a ones
    column in V.  Scores AND the AV matmuls are causally trimmed to
    [qo:512] (skip_group_check for the partial stop flags).  Per-kt
    software pipeline: scores(kt+1) is emitted before AV(kt) so exp(kt)
    runs under PE work.  Normalization broadcasts the sums row with a K=1
    ones matmul + fast reciprocal; normalized ctx^T stays in SBUF ([128,
    512] per group, both heads stacked) and feeds c_proj directly.
  - c_proj: per (b,g): 8 single-shot matmuls (contraction = my 128 ctx
    dims) produce [128 j, 512 m] partials; evacuation alternates DVE/ACT;
    out-DMAs alternate the sync/gpsimd queues.
  - Emission order software-pipelines phases: QKV(b1) and c_proj units
    interleave into the attention PE stream as fill work.
All matmuls bf16 inputs with fp32 PSUM accumulation; partials bf16
(rel err 0.0046 vs the 2e-2 gate).
"""

import numpy as np
import ml_dtypes

import concourse.bass as bass
import concourse.tile as tile
import concourse.mybir as mybir
from concourse import bacc
from concourse.bass_utils import run_bass_kernel_spmd

F32 = mybir.dt.float32
BF16 = mybir.dt.bfloat16

NCORES = 8
B, S, D = 2, 2048, 1024
H, HD = 16, 64
HPC = H // NCORES          # heads per core = 2
M = B * S                  # 4096 rows
ST_B = S // 128            # 16 s-tiles per batch
KCH = D // 128             # 8 contraction chunks
G_PER_B = S // 512         # 4 q-supers per batch
SCALE = 1.0 / (HD ** 0.5)
EXP = mybir.ActivationFunctionType.Exp


def build_ir(nc):
    # ---------------- DRAM I/O ----------------
    # all inputs host-laid-out so every DMA has 128 partition-contiguous
    # descriptors (descriptor generation, ~8ns each, throttles startup
    # otherwise): xt as [p, su, c, m'], weights as [p, c, j]
    xt = nc.dram_tensor("xt", [128, 8 * KCH * 512], BF16,
                        kind="ExternalInput").ap()
    wqk = nc.dram_tensor("wqk", [128, KCH * 256], BF16,
                         kind="ExternalInput").ap()
    wv = nc.dram_tensor("wv", [128, KCH * 128], BF16,
                        kind="ExternalInput").ap()
    wp = nc.dram_tensor("wp", [128, D], BF16, kind="ExternalInput").ap()
    bqk = nc.dram_tensor("bqk", [128, 2], F32, kind="ExternalInput").ap()
    # tiny row [bqk(256) | bv(128)] bf16; broadcast tiles built on-device
    brows = nc.dram_tensor("brows", [1, 384], BF16, kind="ExternalInput").ap()
    outP = nc.dram_tensor("outP", [D, M], BF16, kind="ExternalOutput").ap()

    # causal mask for the diagonal 128-block: mask[k, c] = 1 if c >= k,
    # duplicated for both heads ([128, 2, 128]) so one mul covers a kt tile
    mask_np = (np.arange(128)[None, :] >= np.arange(128)[:, None])
    mask2 = np.stack([mask_np, mask_np], axis=1)
    mask_const = nc.inline_tensor(mask2.astype(ml_dtypes.bfloat16),
                                  "mask_const").ap()

    with tile.TileContext(nc) as tc:
        _emit(nc, tc, xt, wqk, wv, wp, bqk, brows, outP, mask_const)
    return nc


def _emit(nc, tc, xt, wqk, wv, wp, bqk, brows, outP, mask_const):
    import contextlib
    es = contextlib.ExitStack()
    with es:
        singles = es.enter_context(tc.tile_pool(name="singles", bufs=1))

        # ------------- persistent SBUF -------------
        qT = singles.tile([128, M], BF16, tag="qT")
        kT = singles.tile([128, M], BF16, tag="kT")
        v_sb = singles.tile([128, B * ST_B, 130], BF16, tag="v_sb")
        mask_sb = singles.tile([128, 2, 128], BF16, tag="mask_sb")
        wqk_sb = singles.tile([128, KCH, 256], BF16, tag="wqk_sb")
        wv_sb = singles.tile([128, KCH, 128], BF16, tag="wv_sb")
        wp_sb = singles.tile([128, KCH, 128], BF16, tag="wp_sb")
        bqk_sb = singles.tile([128, 2], F32, tag="bqk_sb")
        brows_sb = singles.tile([1, 384], BF16, tag="brows_sb")
        bqk_bc_sb = singles.tile([128, 2, 512], BF16, tag="bqk_bc_sb")
        bv_bc_sb = singles.tile([128, 2, 64], BF16, tag="bv_bc_sb")
        ones_p64 = singles.tile([65, 64], F32, tag="ones_p64")
        ones512 = singles.tile([1, 512], BF16, tag="ones512")

        # input DMAs spread across the three DMA-capable queues (sync,
        # scalar, gpsimd), priority-ordered: the first attention group needs
        # wqk + x^T(su0) + mask + bias rows, so those lead their queues.
        nc.scalar.dma_start(out=brows_sb, in_=brows)
        nc.scalar.dma_start(out=wqk_sb,
                            in_=wqk.rearrange("p (c j) -> p c j", j=256))
        nc.scalar.dma_start(out=bqk_sb, in_=bqk)
        nc.scalar.dma_start(out=wv_sb,
                            in_=wv.rearrange("p (c j) -> p c j", j=128))
        nc.scalar.dma_start(out=mask_sb, in_=mask_const)
        nc.scalar.dma_start(out=wp_sb,
                            in_=wp.rearrange("p (c j) -> p c j", j=128))
        nc.vector.memset(ones512, 1.0)
        nc.vector.memset(ones_p64, 1.0)
        # ones columns of V (cols 64 and 129); data cols are written by evac
        nc.vector.memset(v_sb[:, :, 64:65], 1.0)
        nc.vector.memset(v_sb[:, :, 129:130], 1.0)

        # PE warmup K=1 matmuls while DMAs stream, then build the broadcast
        # bias tiles on-device: outer products of ones and the bias row
        with tc.tile_pool(name="warm_ps", bufs=1, space="PSUM") as warm_ps:
            wt = warm_ps.tile([128, 512], F32)
            for _ in range(8):
                nc.tensor.matmul(wt, lhsT=ones512[:, 0:128],
                                 rhs=ones512, start=True, stop=True)
            for jt in range(2):
                nc.tensor.matmul(wt, lhsT=brows_sb[:, jt * 128:(jt + 1) * 128],
                                 rhs=ones512, start=True, stop=True)
                nc.vector.tensor_copy(bqk_bc_sb[:, jt, :], wt)
            nc.tensor.matmul(wt[:, 0:128], lhsT=ones512[:, 0:128],
                             rhs=brows_sb[:, 256:384], start=True, stop=True)
            for hl in range(HPC):
                nc.vector.tensor_copy(bv_bc_sb[:, hl, :],
                                      wt[:, hl * 64:(hl + 1) * 64])

        # x^T in su-major host layout [p, su, c, m']: each su slice is
        # split into kc-halves delivered on the sync AND gpsimd queues in
        # parallel, in strict consumption order -- each su completes ~2x
        # sooner and the QKV accumulation can begin on the first half
        # (per-region dependency tracking) while the second streams in
        xt_r = xt.rearrange("p (su c m) -> p su c m", su=8, c=KCH)
        xt_sb = singles.tile([128, 8, KCH, 512], BF16, tag="xt_sb")
        for su in range(8):
            nc.sync.dma_start(out=xt_sb[:, su:su + 1, 0:4, :],
                              in_=xt_r[:, su:su + 1, 0:4, :])
            nc.gpsimd.dma_start(out=xt_sb[:, su:su + 1, 4:8, :],
                                in_=xt_r[:, su:su + 1, 4:8, :])

        def xt_cols(c, m0, m1):
            """slice of x^T chunk c for global columns [m0, m1)"""
            su = m0 // 512
            assert m1 <= (su + 1) * 512
            return xt_sb[:, su, c, m0 - su * 512:m1 - su * 512]

        # ------------- shared psum pools (8 banks total) -------------
        s_ps = es.enter_context(tc.tile_pool(name="s_ps", bufs=2, space="PSUM"))
        ctx_ps = es.enter_context(tc.tile_pool(name="ctx_ps", bufs=2, space="PSUM"))
        mm_ps = es.enter_context(tc.tile_pool(name="mm_ps", bufs=2, space="PSUM"))

        pt_pool = es.enter_context(tc.tile_pool(name="pt", bufs=6))
        row_pool = es.enter_context(tc.tile_pool(name="row", bufs=3))
        rec_pool = es.enter_context(tc.tile_pool(name="rec", bufs=3))
        cs_pool = es.enter_context(tc.tile_pool(name="cs", bufs=4))
        osb = es.enter_context(tc.tile_pool(name="osb", bufs=6))

        # ------------- QKV emitters -------------
        def emit_qk(su, jt, use_act=False):
            """Q^T (jt=0) or K^T (jt=1) for row-super su (512 cols)."""
            dst = qT if jt == 0 else kT
            ps = mm_ps.tile([128, 512], F32, tag="mm")
            for kc in range(KCH):
                nc.tensor.matmul(
                    ps,
                    lhsT=wqk_sb[:, kc, jt * 128:(jt + 1) * 128],
                    rhs=xt_cols(kc, su * 512, (su + 1) * 512),
                    start=(kc == 0), stop=(kc == KCH - 1),
                )
            dslice = dst[:, su * 512:(su + 1) * 512]
            if use_act:   # ACT idle in prologue: fused bias-add evacuation
                nc.scalar.add(dslice, ps, bqk_sb[:, jt:jt + 1])
            else:
                nc.vector.tensor_add(dslice, ps, bqk_bc_sb[:, jt, :])

        def emit_v(st):
            """V (row-major) for global s-tile st (128 rows)."""
            ps = mm_ps.tile([128, 512], F32, tag="mm")
            for kc in range(KCH):
                nc.tensor.matmul(
                    ps[:, 0:128],
                    lhsT=xt_cols(kc, st * 128, (st + 1) * 128),
                    rhs=wv_sb[:, kc, :],
                    start=(kc == 0), stop=(kc == KCH - 1),
                )
            for hl in range(HPC):
                nc.vector.tensor_add(
                    v_sb[:, st, hl * 65:hl * 65 + 64],
                    ps[:, hl * 64:(hl + 1) * 64],
                    bv_bc_sb[:, hl, :])

        # ------------- c_proj emitter -------------
        cs_sets = {}

        def emit_cpmm(b, g, jcs, use_act=False):
            """partial out^T rows [jc*128,...), cols [b*S+g*512, +512), from
            my 128 ctx dims.  During attention the evacuations stay on DVE
            (ACT is exp-critical); the post-attention tail also uses ACT."""
            cs = cs_sets[(b, g)]
            col = b * S + g * 512
            for jc in jcs:
                ps = mm_ps.tile([128, 512], F32, tag="mm")
                nc.tensor.matmul(ps, lhsT=wp_sb[:, jc, :], rhs=cs,
                                 start=True, stop=True)
                o = osb.tile([128, 512], BF16, tag="o")
                if use_act and jc % 2 == 1:
                    nc.scalar.mul(o, ps, 1.0)
                else:
                    nc.vector.tensor_copy(o, ps)
                if use_act:   # kernel tail: spread across all three queues
                    eng = (nc.sync, nc.gpsimd, nc.scalar)[jc % 3]
                else:
                    eng = nc.sync if jc % 2 == 0 else nc.gpsimd
                eng.dma_start(
                    out=outP[jc * 128:(jc + 1) * 128, col:col + 512], in_=o)

        # ------------- attention -------------
        def emit_attn(b, fill, target_rows, add_after_g=None, budget0=1024):
            """Attention for batch b.  Per-kt pipeline: scores(kt+1) is
            emitted before AV(kt).  Fill units are (rows, fn) pairs popped
            against a per-kt row budget that tops each kt up to a constant
            TOTAL row count (trimmed diagonal kts get more fill), keeping
            the PE stream uniformly dense; add_after_g[g] units join the
            queue after g's epilogue."""
            budget = budget0
            for g in range(G_PER_B):
                n_kt = 4 * g + 4
                cps = [ctx_ps.tile([65, 512], F32, tag="ctx", name=f"cps{_hl}")
                       for _hl in range(HPC)]
                q_sl = [qT[hl * 64:(hl + 1) * 64,
                           b * S + g * 512:b * S + (g + 1) * 512]
                        for hl in range(HPC)]
                pend_av = None
                for kt in range(n_kt):
                    qo = max(kt - 4 * g, 0) * 128  # causal trim offset
                    sp = s_ps.tile([128, 2, 512], F32, tag="s")
                    pt = pt_pool.tile([128, 2, 512], BF16, tag="pt")
                    for hl in range(HPC):
                        nc.tensor.matmul(
                            sp[:, hl, qo:512],
                            lhsT=kT[hl * 64:(hl + 1) * 64,
                                    b * S + kt * 128:b * S + (kt + 1) * 128],
                            rhs=q_sl[hl][:, qo:512],
                            start=True, stop=True,
                            tile_position=(64 * hl, 0),
                        )
                    nc.scalar.activation(pt[:, :, qo:512], sp[:, :, qo:512],
                                         EXP, scale=SCALE)
                    if kt >= 4 * g:   # diagonal block mask, both heads
                        nc.vector.tensor_mul(
                            pt[:, :, qo:qo + 128], pt[:, :, qo:qo + 128],
                            mask_sb)
                    if pend_av is not None:
                        pend_av()
                    def av(kt=kt, pt=pt, qo=qo):
                        for hl in range(HPC):
                            nc.tensor.matmul(
                                cps[hl][:, qo:512],
                                lhsT=v_sb[:, b * ST_B + kt,
                                          hl * 65:hl * 65 + 65],
                                rhs=pt[:, hl, qo:512],
                                start=(kt == 0), stop=(kt == n_kt - 1),
                                skip_group_check=True,
                            )
                    pend_av = av
                    budget = min(budget + target_rows, 8192)
                    while fill and budget >= fill[0][0]:
                        rows, fn = fill.pop(0)
                        budget -= rows
                        fn()
                pend_av()
                # per-g normalize: copy the sums row (on ACT -- keeps the
                # DVE queue short), broadcast it across partitions with a
                # K=1 ones matmul, fast reciprocal, then scale ctx out of
                # PSUM into the stacked [128, 512] SBUF tile (head hl on
                # partitions hl*64..) that feeds c_proj; the two heads'
                # chains are interleaved so they overlap across engines
                cs = cs_pool.tile([128, 512], BF16, tag="cs")
                rows_ = [row_pool.tile([65, 512], F32, tag="row",
                                       name=f"row{_hl}") for _hl in range(HPC)]
                bcs = [mm_ps.tile([128, 512], F32, tag="mm", name=f"bc{_hl}")
                       for _hl in range(HPC)]
                recs = [rec_pool.tile([64, 512], F32, tag="rec",
                                      name=f"rec{_hl}") for _hl in range(HPC)]
                for hl in range(HPC):
                    nc.scalar.mul(rows_[hl][64:65, :], cps[hl][64:65, :], 1.0)
                for hl in range(HPC):
                    nc.tensor.matmul(bcs[hl][0:64, :], lhsT=ones_p64[64:65, :],
                                     rhs=rows_[hl][64:65, :], start=True,
                                     stop=True, tile_position=(64, 0))
                for hl in range(HPC):
                    nc.vector.reciprocal_approx_fast(recs[hl], bcs[hl][0:64, :])
                    nc.vector.tensor_mul(cs[hl * 64:(hl + 1) * 64, :],
                                         cps[hl][0:64, :], recs[hl])
                cs_sets[(b, g)] = cs
                if add_after_g and g in add_after_g:
                    fill.extend(add_after_g[g])
            return fill

        # ------------- choreography -------------
        # QKV prologue: everything attention(b0) g0 and g1's start need
        # (su0+su1 Q/K and v0-3), ACT evacuations while ACT is idle
        for su in range(2):
            emit_qk(su, 0, use_act=True)
            emit_qk(su, 1, use_act=True)
        for st in range(4):
            emit_v(st)

        # fill units (rows, fn) in deadline order; the per-kt row budget
        # spreads them uniformly so no zone of the PE stream runs dry
        # kT evacuations ride ACT (per-partition bias add) to keep the DVE
        # queue short -- mm_ps turnaround was the top mid-kernel PE blocker
        qk_u = lambda su, jt: (4096, lambda: emit_qk(su, jt, use_act=(jt == 1)))
        v_u = lambda st: (1024, lambda: emit_v(st))
        mm_u = lambda b, g, half: (
            2048, lambda: emit_cpmm(b, g, range(half * 4, half * 4 + 4)))

        fill = [v_u(st) for st in range(4, 8)]
        for su in range(2, 5):
            fill.append(qk_u(su, 0))
            fill.append(qk_u(su, 1))
            for st in range(su * 4, su * 4 + 4):
                fill.append(v_u(st))

        after0 = {0: [mm_u(0, 0, 0), mm_u(0, 0, 1)],
                  1: [mm_u(0, 1, 0), mm_u(0, 1, 1)],
                  2: [mm_u(0, 2, 0), mm_u(0, 2, 1)]}
        fill = emit_attn(0, fill, 1450, add_after_g=after0)

        # b1 stream: su5-7 QKV in deadline order, then c_proj units
        fill2 = fill  # b0 leftovers first
        fill2.append(qk_u(5, 0))
        fill2.append(qk_u(5, 1))
        fill2.extend(v_u(st) for st in range(20, 24))
        fill2.append(qk_u(6, 0))
        fill2.append(qk_u(6, 1))
        fill2.extend(v_u(st) for st in range(24, 28))
        fill2.append(qk_u(7, 0))
        fill2.append(qk_u(7, 1))
        fill2.extend(v_u(st) for st in range(28, 32))
        fill2.append(mm_u(0, 3, 0))
        fill2.append(mm_u(0, 3, 1))
        after1 = {0: [mm_u(1, 0, 0), mm_u(1, 0, 1)],
                  1: [mm_u(1, 1, 0), mm_u(1, 1, 1)],
                  2: [mm_u(1, 2, 0), mm_u(1, 2, 1)]}
        fill2 = emit_attn(1, fill2, 1500, add_after_g=after1)
        for _, fn in fill2:   # leftovers
            fn()
        emit_cpmm(1, 3, range(KCH), use_act=True)


_CACHE = {}


def _get_compiled():
    if "nc" not in _CACHE:
        nc = bacc.Bacc("TRN2", target_bir_lowering=False, debug=False,
                       num_devices=NCORES)
        build_ir(nc)
        nc.compile()
        _CACHE["nc"] = nc
    return _CACHE["nc"]


def make_in_maps(inputs):
    x = np.asarray(inputs["hidden_states"], dtype=np.float32)   # [B,S,D]
    wa = np.asarray(inputs["c_attn_w"], dtype=np.float32)       # [D, 3D]
    ba = np.asarray(inputs["c_attn_b"], dtype=np.float32)       # [3D]
    wpr = np.asarray(inputs["c_proj_w"], dtype=np.float32)      # [D, D]

    bf = ml_dtypes.bfloat16
    xT = np.ascontiguousarray(x.reshape(M, D).T).astype(bf)     # [D, M]
    wq, wk, wv_full = wa[:, 0:D], wa[:, D:2 * D], wa[:, 2 * D:3 * D]
    bq, bk, bv_full = ba[0:D], ba[D:2 * D], ba[2 * D:3 * D]

    in_maps = []
    for r in range(NCORES):
        hs = slice(r * HPC * HD, (r + 1) * HPC * HD)   # this core's head dims
        bqk_r = np.concatenate([bq[hs], bk[hs]])       # [256]
        bv_r = bv_full[hs]                             # [128]
        wqk_r = np.concatenate([wq[:, hs], wk[:, hs]], axis=1)  # [D, 256]
        in_maps.append({
            # [p, su, c, m']: partition-contiguous su-blocks of x^T
            "xt": np.ascontiguousarray(
                xT.reshape(KCH, 128, 8, 512).transpose(1, 2, 0, 3)
                .reshape(128, 8 * KCH * 512)),
            "wqk": np.ascontiguousarray(
                wqk_r.reshape(KCH, 128, 256).transpose(1, 0, 2)
                .reshape(128, KCH * 256)).astype(bf),
            "wv": np.ascontiguousarray(
                wv_full[:, hs].reshape(KCH, 128, 128).transpose(1, 0, 2)
                .reshape(128, KCH * 128)).astype(bf),
            "wp": np.ascontiguousarray(wpr[hs, :]).astype(bf),
            "bqk": np.ascontiguousarray(bqk_r.reshape(2, 128).T),
            "brows": np.ascontiguousarray(np.concatenate(
                [bqk_r, bv_r]).reshape(1, 384)).astype(bf),
        })
    return in_maps


def assemble(results, c_proj_b):
    acc = results[0]["outP"].astype(np.float32)
    for r in range(1, NCORES):
        acc = acc + results[r]["outP"].astype(np.float32)
    out = acc.T.reshape(B, S, D) + c_proj_b[None, None, :]
    return np.ascontiguousarray(out.astype(np.float32))


def kernel(**inputs):
    in_maps = make_in_maps(inputs)
    nc = _get_compiled()
    res = run_bass_kernel_spmd(nc, in_maps, core_ids=list(range(NCORES)))
    return assemble(res.results,
                    np.asarray(inputs["c_proj_b"], dtype=np.float32))


if __name__ == "__main__":
    import reference
    inp = reference.setup_inputs()
    out = kernel(**{k: np.asarray(v) for k, v in inp.items()})
    print(out.shape, out.dtype)


# revision 61
# speedup vs baseline: 1.0271x; 1.0271x over previous
"""Trainium2 Bass kernel for ClassicAttention (B=2, S=2048, D=1024, H=16).

Sharding: tensor-parallel over heads across 8 cores (2 heads/core), with
NO on-device collectives: each core computes a partial c_proj output from
its own heads' context (c_proj input rows are head dims, per the TP-head
sharding), DMAs the [1024, M] bf16 partial to DRAM, and the host sums the
8 partials during unsharding.  Profiling showed the collective engine
costs ~30us per ctx AllGather when overlapped with compute (~120us total)
plus an exposed tail; the partial-sum form replaces all of that with
~25us of fully-overlapped output DMA and a small tail.

  - Host pre-transposes x to x^T [D, M] and pre-casts all matmul operands
    to bf16; broadcast bias tiles are built on-device with K=1 matmuls from
    a 1KB bias row (no seed matmuls in the steady state, bias fused into
    PSUM-evacuation adds).
  - QKV: each core computes Q^T,K^T (d-major) and V (row-major) for its 2
    heads over all B*S rows straight from x^T in SBUF.
  - Attention: transposed-scores formulation S^T[k,q]; both heads share one
    [128,1024] score tile so each k-tile needs a single exp ACTIVATE.  The
    softmax denominator rides row 64 of the AV accumulator via a ones
    column in V.  Scores AND the AV matmuls are causally trimmed to
    [qo:512] (skip_group_check for the partial stop flags).  Per-kt
    software pipeline: scores(kt+1) is emitted before AV(kt) so exp(kt)
    runs under PE work.  Normalization broadcasts the sums row with a K=1
    ones matmul + fast reciprocal; normalized ctx^T stays in SBUF ([128,
    512] per group, both heads stacked) and feeds c_proj directly.
  - c_proj: per (b,g): 8 single-shot matmuls (contraction = my 128 ctx
    dims) produce [128 j, 512 m] partials; evacuation alternates DVE/ACT;
    out-DMAs alternate the sync/gpsimd queues.
  - Emission order software-pipelines phases: QKV(b1) and c_proj units
    interleave into the attention PE stream as fill work.
All matmuls bf16 inputs with fp32 PSUM accumulation; partials bf16
(rel err 0.0046 vs the 2e-2 gate).
"""

import numpy as np
import ml_dtypes

import concourse.bass as bass
import concourse.tile as tile
import concourse.mybir as mybir
from concourse import bacc
from concourse.bass_utils import run_bass_kernel_spmd

F32 = mybir.dt.float32
BF16 = mybir.dt.bfloat16

NCORES = 8
B, S, D = 2, 2048, 1024
H, HD = 16, 64
HPC = H // NCORES          # heads per core = 2
M = B * S                  # 4096 rows
ST_B = S // 128            # 16 s-tiles per batch
KCH = D // 128             # 8 contraction chunks
G_PER_B = S // 512         # 4 q-supers per batch
SCALE = 1.0 / (HD ** 0.5)
EXP = mybir.ActivationFunctionType.Exp


def build_ir(nc):
    # ---------------- DRAM I/O ----------------
    # all inputs host-laid-out so every DMA has 128 partition-contiguous
    # descriptors (descriptor generation, ~8ns each, throttles startup
    # otherwise): xt as [p, su, c, m'], weights as [p, c, j]
    xt = nc.dram_tensor("xt", [128, 8 * KCH * 512], BF16,
                        kind="ExternalInput").ap()
    wqk = nc.dram_tensor("wqk", [128, KCH * 256], BF16,
                         kind="ExternalInput").ap()
    wv = nc.dram_tensor("wv", [128, KCH * 128], BF16,
                        kind="ExternalInput").ap()
    wp = nc.dram_tensor("wp", [128, D], BF16, kind="ExternalInput").ap()
    bqk = nc.dram_tensor("bqk", [128, 2], F32, kind="ExternalInput").ap()
    # tiny row [bqk(256) | bv(128)] bf16; broadcast tiles built on-device
    brows = nc.dram_tensor("brows", [1, 384], BF16, kind="ExternalInput").ap()
    outP = nc.dram_tensor("outP", [D, M], BF16, kind="ExternalOutput").ap()

    # causal mask for the diagonal 128-block: mask[k, c] = 1 if c >= k,
    # duplicated for both heads ([128, 2, 128]) so one mul covers a kt tile
    mask_np = (np.arange(128)[None, :] >= np.arange(128)[:, None])
    mask2 = np.stack([mask_np, mask_np], axis=1)
    mask_const = nc.inline_tensor(mask2.astype(ml_dtypes.bfloat16),
                                  "mask_const").ap()

    with tile.TileContext(nc) as tc:
        _emit(nc, tc, xt, wqk, wv, wp, bqk, brows, outP, mask_const)
    return nc


def _emit(nc, tc, xt, wqk, wv, wp, bqk, brows, outP, mask_const):
    import contextlib
    es = contextlib.ExitStack()
    with es:
        singles = es.enter_context(tc.tile_pool(name="singles", bufs=1))

        # ------------- persistent SBUF -------------
        qT = singles.tile([128, M], BF16, tag="qT")
        kT = singles.tile([128, M], BF16, tag="kT")
        v_sb = singles.tile([128, B * ST_B, 130], BF16, tag="v_sb")
        mask_sb = singles.tile([128, 2, 128], BF16, tag="mask_sb")
        wqk_sb = singles.tile([128, KCH, 256], BF16, tag="wqk_sb")
        wv_sb = singles.tile([128, KCH, 128], BF16, tag="wv_sb")
        wp_sb = singles.tile([128, KCH, 128], BF16, tag="wp_sb")
        bqk_sb = singles.tile([128, 2], F32, tag="bqk_sb")
        brows_sb = singles.tile([1, 384], BF16, tag="brows_sb")
        bqk_bc_sb = singles.tile([128, 2, 512], BF16, tag="bqk_bc_sb")
        bv_bc_sb = singles.tile([128, 2, 64], BF16, tag="bv_bc_sb")
        ones_p64 = singles.tile([65, 64], F32, tag="ones_p64")
        ones512 = singles.tile([1, 512], BF16, tag="ones512")

        # input DMAs spread across the three DMA-capable queues (sync,
        # scalar, gpsimd), priority-ordered: the first attention group needs
        # wqk + x^T(su0) + mask + bias rows, so those lead their queues.
        nc.scalar.dma_start(out=brows_sb, in_=brows)
        nc.scalar.dma_start(out=wqk_sb,
                            in_=wqk.rearrange("p (c j) -> p c j", j=256))
        nc.scalar.dma_start(out=bqk_sb, in_=bqk)
        nc.scalar.dma_start(out=wv_sb,
                            in_=wv.rearrange("p (c j) -> p c j", j=128))
        nc.scalar.dma_start(out=mask_sb, in_=mask_const)
        nc.scalar.dma_start(out=wp_sb,
                            in_=wp.rearrange("p (c j) -> p c j", j=128))
        nc.vector.memset(ones512, 1.0)
        nc.vector.memset(ones_p64, 1.0)
        # ones columns of V (cols 64 and 129); data cols are written by evac
        nc.vector.memset(v_sb[:, :, 64:65], 1.0)
        nc.vector.memset(v_sb[:, :, 129:130], 1.0)

        # PE warmup K=1 matmuls while DMAs stream, then build the broadcast
        # bias tiles on-device: outer products of ones and the bias row
        with tc.tile_pool(name="warm_ps", bufs=1, space="PSUM") as warm_ps:
            wt = warm_ps.tile([128, 512], F32)
            for _ in range(8):
                nc.tensor.matmul(wt, lhsT=ones512[:, 0:128],
                                 rhs=ones512, start=True, stop=True)
            for jt in range(2):
                nc.tensor.matmul(wt, lhsT=brows_sb[:, jt * 128:(jt + 1) * 128],
                                 rhs=ones512, start=True, stop=True)
                nc.vector.tensor_copy(bqk_bc_sb[:, jt, :], wt)
            nc.tensor.matmul(wt[:, 0:128], lhsT=ones512[:, 0:128],
                             rhs=brows_sb[:, 256:384], start=True, stop=True)
            for hl in range(HPC):
                nc.vector.tensor_copy(bv_bc_sb[:, hl, :],
                                      wt[:, hl * 64:(hl + 1) * 64])

        # x^T in su-major host layout [p, su, c, m']: each su slice is
        # split into kc-halves delivered on the sync AND gpsimd queues in
        # parallel, in strict consumption order -- each su completes ~2x
        # sooner and the QKV accumulation can begin on the first half
        # (per-region dependency tracking) while the second streams in
        xt_r = xt.rearrange("p (su c m) -> p su c m", su=8, c=KCH)
        xt_sb = singles.tile([128, 8, KCH, 512], BF16, tag="xt_sb")
        for su in range(8):
            nc.sync.dma_start(out=xt_sb[:, su:su + 1, 0:4, :],
                              in_=xt_r[:, su:su + 1, 0:4, :])
            nc.gpsimd.dma_start(out=xt_sb[:, su:su + 1, 4:8, :],
                                in_=xt_r[:, su:su + 1, 4:8, :])

        def xt_cols(c, m0, m1):
            """slice of x^T chunk c for global columns [m0, m1)"""
            su = m0 // 512
            assert m1 <= (su + 1) * 512
            return xt_sb[:, su, c, m0 - su * 512:m1 - su * 512]

        # ------------- shared psum pools (8 banks total) -------------
        s_ps = es.enter_context(tc.tile_pool(name="s_ps", bufs=2, space="PSUM"))
        ctx_ps = es.enter_context(tc.tile_pool(name="ctx_ps", bufs=2, space="PSUM"))
        mm_ps = es.enter_context(tc.tile_pool(name="mm_ps", bufs=2, space="PSUM"))

        pt_pool = es.enter_context(tc.tile_pool(name="pt", bufs=6))
        row_pool = es.enter_context(tc.tile_pool(name="row", bufs=3))
        rec_pool = es.enter_context(tc.tile_pool(name="rec", bufs=3))
        cs_pool = es.enter_context(tc.tile_pool(name="cs", bufs=4))
        osb = es.enter_context(tc.tile_pool(name="osb", bufs=6))

        # ------------- QKV emitters -------------
        def emit_qk(su, jt, use_act=False):
            """Q^T (jt=0) or K^T (jt=1) for row-super su (512 cols)."""
            dst = qT if jt == 0 else kT
            ps = mm_ps.tile([128, 512], F32, tag="mm")
            for kc in range(KCH):
                nc.tensor.matmul(
                    ps,
                    lhsT=wqk_sb[:, kc, jt * 128:(jt + 1) * 128],
                    rhs=xt_cols(kc, su * 512, (su + 1) * 512),
                    start=(kc == 0), stop=(kc == KCH - 1),
                )
            dslice = dst[:, su * 512:(su + 1) * 512]
            if use_act:   # ACT idle in prologue: fused bias-add evacuation
                nc.scalar.add(dslice, ps, bqk_sb[:, jt:jt + 1])
            else:
                nc.vector.tensor_add(dslice, ps, bqk_bc_sb[:, jt, :])

        def emit_v(st):
            """V (row-major) for global s-tile st (128 rows)."""
            ps = mm_ps.tile([128, 512], F32, tag="mm")
            for kc in range(KCH):
                nc.tensor.matmul(
                    ps[:, 0:128],
                    lhsT=xt_cols(kc, st * 128, (st + 1) * 128),
                    rhs=wv_sb[:, kc, :],
                    start=(kc == 0), stop=(kc == KCH - 1),
                )
            for hl in range(HPC):
                nc.vector.tensor_add(
                    v_sb[:, st, hl * 65:hl * 65 + 64],
                    ps[:, hl * 64:(hl + 1) * 64],
                    bv_bc_sb[:, hl, :])

        # ------------- c_proj emitter -------------
        cs_sets = {}

        def emit_cpmm(b, g, jcs, use_act=False):
            """partial out^T rows [jc*128,...), cols [b*S+g*512, +512), from
            my 128 ctx dims.  During attention the evacuations stay on DVE
            (ACT is exp-critical); the post-attention tail also uses ACT."""
            cs = cs_sets[(b, g)]
            col = b * S + g * 512
            for jc in jcs:
                ps = mm_ps.tile([128, 512], F32, tag="mm")
                nc.tensor.matmul(ps, lhsT=wp_sb[:, jc, :], rhs=cs,
                                 start=True, stop=True)
                o = osb.tile([128, 512], BF16, tag="o")
                if use_act and jc % 2 == 1:
                    nc.scalar.mul(o, ps, 1.0)
                else:
                    nc.vector.tensor_copy(o, ps)
                if use_act:   # kernel tail: spread across all three queues
                    eng = (nc.sync, nc.gpsimd, nc.scalar)[jc % 3]
                else:
                    eng = nc.sync if jc % 2 == 0 else nc.gpsimd
                eng.dma_start(
                    out=outP[jc * 128:(jc + 1) * 128, col:col + 512], in_=o)

        # ------------- attention -------------
        def emit_attn(b, fill, target_rows, add_after_g=None, budget0=1024):
            """Attention for batch b.  Per-kt pipeline: scores(kt+1) is
            emitted before AV(kt).  Fill units are (rows, fn) pairs popped
            against a per-kt row budget that tops each kt up to a constant
            TOTAL row count (trimmed diagonal kts get more fill), keeping
            the PE stream uniformly dense; add_after_g[g] units join the
            queue after g's epilogue."""
            budget = budget0
            for g in range(G_PER_B):
                n_kt = 4 * g + 4
                cps = [ctx_ps.tile([65, 512], F32, tag="ctx", name=f"cps{_hl}")
                       for _hl in range(HPC)]
                q_sl = [qT[hl * 64:(hl + 1) * 64,
                           b * S + g * 512:b * S + (g + 1) * 512]
                        for hl in range(HPC)]
                pend_av = None
                for kt in range(n_kt):
                    qo = max(kt - 4 * g, 0) * 128  # causal trim offset
                    sp = s_ps.tile([128, 2, 512], F32, tag="s")
                    pt = pt_pool.tile([128, 2, 512], BF16, tag="pt")
                    for hl in range(HPC):
                        nc.tensor.matmul(
                            sp[:, hl, qo:512],
                            lhsT=kT[hl * 64:(hl + 1) * 64,
                                    b * S + kt * 128:b * S + (kt + 1) * 128],
                            rhs=q_sl[hl][:, qo:512],
                            start=True, stop=True,
                            tile_position=(64 * hl, 0),
                        )
                    nc.scalar.activation(pt[:, :, qo:512], sp[:, :, qo:512],
                                         EXP, scale=SCALE)
                    if kt >= 4 * g:   # diagonal block mask, both heads
                        nc.vector.tensor_mul(
                            pt[:, :, qo:qo + 128], pt[:, :, qo:qo + 128],
                            mask_sb)
                    if pend_av is not None:
                        pend_av()
                    def av(kt=kt, pt=pt, qo=qo):
                        for hl in range(HPC):
                            nc.tensor.matmul(
                                cps[hl][:, qo:512],
                                lhsT=v_sb[:, b * ST_B + kt,
                                          hl * 65:hl * 65 + 65],
                                rhs=pt[:, hl, qo:512],
                                start=(kt == 0), stop=(kt == n_kt - 1),
                                skip_group_check=True,
                            )
                    pend_av = av
                    budget = min(budget + target_rows, 8192)
                    while fill and budget >= fill[0][0]:
                        rows, fn = fill.pop(0)
                        budget -= rows
                        fn()
                pend_av()
                # per-g normalize: copy the sums row (on ACT -- keeps the
                # DVE queue short), broadcast it across partitions with a
                # K=1 ones matmul, fast reciprocal, then scale ctx out of
                # PSUM into the stacked [128, 512] SBUF tile (head hl on
                # partitions hl*64..) that feeds c_proj; the two heads'
                # chains are interleaved so they overlap across engines
                cs = cs_pool.tile([128, 512], BF16, tag="cs")
                rows_ = [row_pool.tile([65, 512], F32, tag="row",
                                       name=f"row{_hl}") for _hl in range(HPC)]
                bcs = [mm_ps.tile([128, 512], F32, tag="mm", name=f"bc{_hl}")
                       for _hl in range(HPC)]
                recs = [rec_pool.tile([64, 512], F32, tag="rec",
                                      name=f"rec{_hl}") for _hl in range(HPC)]
                for hl in range(HPC):
                    nc.scalar.mul(rows_[hl][64:65, :], cps[hl][64:65, :], 1.0)
                for hl in range(HPC):
                    nc.tensor.matmul(bcs[hl][0:64, :], lhsT=ones_p64[64:65, :],
                                     rhs=rows_[hl][64:65, :], start=True,
                                     stop=True, tile_position=(64, 0))
                for hl in range(HPC):
                    nc.vector.reciprocal_approx_fast(recs[hl], bcs[hl][0:64, :])
                    nc.vector.tensor_mul(cs[hl * 64:(hl + 1) * 64, :],
                                         cps[hl][0:64, :], recs[hl])
                cs_sets[(b, g)] = cs
                if add_after_g and g in add_after_g:
                    fill.extend(add_after_g[g])
            return fill

        # ------------- choreography -------------
        # QKV prologue: everything attention(b0) g0 and g1's start need
        # (su0+su1 Q/K and v0-3), ACT evacuations while ACT is idle
        for su in range(2):
            emit_qk(su, 0, use_act=True)
            emit_qk(su, 1, use_act=True)
        for st in range(4):
            emit_v(st)

        # fill units (rows, fn) in deadline order; the per-kt row budget
        # spreads them uniformly so no zone of the PE stream runs dry
        qk_u = lambda su, jt: (4096, lambda: emit_qk(su, jt))
        v_u = lambda st: (1024, lambda: emit_v(st))
        mm_u = lambda b, g, half: (
            2048, lambda: emit_cpmm(b, g, range(half * 4, half * 4 + 4)))

        fill = [v_u(st) for st in range(4, 8)]
        for su in range(2, 5):
            fill.append(qk_u(su, 0))
            fill.append(qk_u(su, 1))
            for st in range(su * 4, su * 4 + 4):
                fill.append(v_u(st))

        after0 = {0: [mm_u(0, 0, 0), mm_u(0, 0, 1)],
                  1: [mm_u(0, 1, 0), mm_u(0, 1, 1)],
                  2: [mm_u(0, 2, 0), mm_u(0, 2, 1)]}
        fill = emit_attn(0, fill, 1450, add_after_g=after0)

        # b1 stream: su5-7 QKV in deadline order, then c_proj units
        fill2 = fill  # b0 leftovers first
        fill2.append(qk_u(5, 0))
        fill2.append(qk_u(5, 1))
        fill2.extend(v_u(st) for st in range(20, 24))
        fill2.append(qk_u(6, 0))
        fill2.append(qk_u(6, 1))
        fill2.extend(v_u(st) for st in range(24, 28))
        fill2.append(qk_u(7, 0))
        fill2.append(qk_u(7, 1))
        fill2.extend(v_u(st) for st in range(28, 32))
        fill2.append(mm_u(0, 3, 0))
        fill2.append(mm_u(0, 3, 1))
        after1 = {0: [mm_u(1, 0, 0), mm_u(1, 0, 1)],
                  1: [mm_u(1, 1, 0), mm_u(1, 1, 1)],
                  2: [mm_u(1, 2, 0), mm_u(1, 2, 1)]}
        fill2 = emit_attn(1, fill2, 1500, add_after_g=after1)
        for _, fn in fill2:   # leftovers
            fn()
        emit_cpmm(1, 3, range(KCH), use_act=True)


_CACHE = {}


def _get_compiled():
    if "nc" not in _CACHE:
        nc = bacc.Bacc("TRN2", target_bir_lowering=False, debug=False,
                       num_devices=NCORES)
        build_ir(nc)
        nc.compile()
        _CACHE["nc"] = nc
    return _CACHE["nc"]


def make_in_maps(inputs):
    x = np.asarray(inputs["hidden_states"], dtype=np.float32)   # [B,S,D]
    wa = np.asarray(inputs["c_attn_w"], dtype=np.float32)       # [D, 3D]
    ba = np.asarray(inputs["c_attn_b"], dtype=np.float32)       # [3D]
    wpr = np.asarray(inputs["c_proj_w"], dtype=np.float32)      # [D, D]

    bf = ml_dtypes.bfloat16
    xT = np.ascontiguousarray(x.reshape(M, D).T).astype(bf)     # [D, M]
    wq, wk, wv_full = wa[:, 0:D], wa[:, D:2 * D], wa[:, 2 * D:3 * D]
    bq, bk, bv_full = ba[0:D], ba[D:2 * D], ba[2 * D:3 * D]

    in_maps = []
    for r in range(NCORES):
        hs = slice(r * HPC * HD, (r + 1) * HPC * HD)   # this core's head dims
        bqk_r = np.concatenate([bq[hs], bk[hs]])       # [256]
        bv_r = bv_full[hs]                             # [128]
        wqk_r = np.concatenate([wq[:, hs], wk[:, hs]], axis=1)  # [D, 256]
        in_maps.append({
            # [p, su, c, m']: partition-contiguous su-blocks of x^T
            "xt": np.ascontiguousarray(
                xT.reshape(KCH, 128, 8, 512).transpose(1, 2, 0, 3)
                .reshape(128, 8 * KCH * 512)),
            "wqk": np.ascontiguousarray(
                wqk_r.reshape(KCH, 128, 256).transpose(1, 0, 2)
                .reshape(128, KCH * 256)).astype(bf),
            "wv": np.ascontiguousarray(
                wv_full[:, hs].reshape(KCH, 128, 128).transpose(1, 0, 2)
                .reshape(128, KCH * 128)).astype(bf),
            "wp": np.ascontiguousarray(wpr[hs, :]).astype(bf),
            "bqk": np.ascontiguousarray(bqk_r.reshape(2, 128).T),
            "brows": np.ascontiguousarray(np.concatenate(
                [bqk_r, bv_r]).reshape(1, 384)).astype(bf),
        })
    return in_maps


def assemble(results, c_proj_b):
    acc = results[0]["outP"].astype(np.float32)
    for r in range(1, NCORES):
        acc = acc + results[r]["outP"].astype(np.float32)
    out = acc.T.reshape(B, S, D) + c_proj_b[None, None, :]
    return np.ascontiguousarray(out.astype(np.float32))


def kernel(**inputs):
    in_maps = make_in_maps(inputs)
    nc = _get_compiled()
    res = run_bass_kernel_spmd(nc, in_maps, core_ids=list(range(NCORES)))
    return assemble(res.results,
                    np.asarray(inputs["c_proj_b"], dtype=np.float32))


if __name__ == "__main__":
    import reference
    inp = reference.setup_inputs()
    out = kernel(**{k: np.asarray(v) for k, v in inp.items()})
    print(out.shape, out.dtype)
